# revision 6
# baseline (speedup 1.0000x reference)
import functools

import jax
import jax.numpy as jnp
import numpy as np

NH, WIN = 8, 7  # num_head, window
B, H, W, C = 8, 112, 112, 128
NDEV = 8


def _ln(x, w, b, eps=1e-6):
    mu = x.mean(-1, keepdims=True)
    var = ((x - mu) ** 2).mean(-1, keepdims=True)
    return (x - mu) / jnp.sqrt(var + eps) * w + b


def _dwconv_shifts(x, w49, b):
    # x: (H,W,Cc) single image, channels-last; w49: (Cc, 7, 7) depthwise; pad 3.
    # Implemented as 49 shifted adds on a zero-padded tensor so the Neuron
    # compiler sees only slices/multiplies/adds (no conv HLO).
    Hh, Ww, Cc = x.shape
    xp = jnp.pad(x, ((3, 3), (3, 3), (0, 0)))
    out = jnp.zeros_like(x)
    for dy in range(7):
        for dx in range(7):
            out = out + xp[dy:dy + Hh, dx:dx + Ww, :] * w49[:, dy, dx]
    return out + b


def _bilinear_mat(n_out, n_in):
    # Half-pixel-center bilinear interpolation matrix (n_out, n_in), matching
    # jax.image.resize(method='bilinear') for upsampling.
    m = np.zeros((n_out, n_in), np.float32)
    scale = n_in / n_out
    for i in range(n_out):
        src = (i + 0.5) * scale - 0.5
        i0 = int(np.floor(src))
        f = src - i0
        i0c = min(max(i0, 0), n_in - 1)
        i1c = min(max(i0 + 1, 0), n_in - 1)
        m[i, i0c] += 1.0 - f
        m[i, i1c] += f
    return m


_UP = jnp.asarray(_bilinear_mat(H, WIN))  # (112, 7)


def _forward_one(x, x_e, p):
    # x: (H,W,C), x_e: (H,W,C/2) — one image.
    hd = C // NH // 2
    xn = _ln(x, p['norm_w'], p['norm_b'])
    xen = _ln(x_e, p['norme_w'], p['norme_b'])

    # fused wide GEMM for all xn consumers: rr1|rr2|rgb|kv (128->448)
    big_w = jnp.concatenate([p['rr1_w'], p['rr2_w'], p['la1_w'], p['kv_w']], 1)
    big_b = jnp.concatenate([p['rr1_b'], p['rr2_b'], p['la1_b'], p['kv_b']])
    big = xn @ big_w + big_b
    rr1, rr2, rgb, kvl = jnp.split(big, [C, 2 * C, 2 * C + C // 2], -1)
    t = xen @ p['la2_w'] + p['la2_b']       # (H,W,C/2)

    # all three depthwise convs as one 256-channel shift pass
    cin = jnp.concatenate([rr2, rgb * t, jnp.abs(rgb - t)], -1)
    cw = jnp.concatenate(
        [p['conv_w'][:, 0], p['lac1_w'][:, 0], p['lac2_w'][:, 0]], 0)
    cb = jnp.concatenate([p['conv_b'], p['lac1_b'], p['lac2_b']])
    cout = _dwconv_shifts(cin, cw, cb)
    rr2c, co_di = cout[:, :, :C], cout[:, :, C:]  # (H,W,C) each

    rr2f = rr2c @ p['rr3_w'] + p['rr3_b']
    local_rr = rr1 * rr2f  # (H,W,C)
    amap = co_di.mean(-1, keepdims=True)    # (H,W,1)
    dot = (amap * co_di).sum((0, 1))        # (C,)
    n1 = jnp.sqrt((amap ** 2).sum())
    n2 = jnp.sqrt((co_di ** 2).sum((0, 1)))  # (C,)
    cos = dot / (n1 * n2 + 1e-6)            # (C,)
    attc = jax.nn.sigmoid(
        jax.nn.gelu(cos @ p['fc1_w'], approximate=False) @ p['fc2_w'])
    local_rx = (co_di * attc) @ p['la3_w'] + p['la3_b']  # (H,W,C/2)

    # global pooled attention
    kv = kvl.reshape(H * W, 2, NH, hd)
    k = kv[:, 0].transpose(1, 0, 2)  # (NH, HW, hd)
    v = kv[:, 1].transpose(1, 0, 2)
    rx = jnp.concatenate([xn, xen], -1)  # (H,W,3C/2)
    ph, pw = H // WIN, W // WIN
    rx_pool = rx.reshape(WIN, ph, WIN, pw, rx.shape[-1]).mean((1, 3))
    q = (rx_pool @ p['q_w'] + p['q_b']).reshape(WIN * WIN, NH, hd)
    q = q.transpose(1, 0, 2)  # (NH, 49, hd)
    logits = (q * hd ** -0.5) @ k.transpose(0, 2, 1)  # (NH, 49, HW)
    logits = logits - logits.max(-1, keepdims=True)
    e = jnp.exp(logits)
    attn = e / e.sum(-1, keepdims=True)
    g = attn @ v  # (NH, 49, hd)
    g = g.reshape(NH, WIN, WIN, hd).transpose(0, 3, 1, 2).reshape(C // 2, WIN, WIN)
    # bilinear 7->112 as two small matmuls: (C/2,112,112)
    g = jnp.einsum('ha,cab,wb->hwc', _UP, g, _UP)

    xc = jnp.concatenate([local_rr, g, local_rx], -1)  # (H,W,2C)
    out_e = xc @ p['proje_w'] + p['proje_b']
    out = xc @ p['proj_w'] + p['proj_b']
    return out, out_e


WEIGHT_KEYS = [
    'norm_w', 'norm_b', 'norme_w', 'norme_b',
    'rr1_w', 'rr1_b', 'rr2_w', 'rr2_b', 'rr3_w', 'rr3_b', 'conv_w', 'conv_b',
    'la1_w', 'la1_b', 'la2_w', 'la2_b', 'lac1_w', 'lac1_b', 'lac2_w', 'lac2_b',
    'fc1_w', 'fc2_w', 'la3_w', 'la3_b',
    'kv_w', 'kv_b', 'q_w', 'q_b', 'proj_w', 'proj_b', 'proje_w', 'proje_b',
]


DEVICES = jax.devices()[:NDEV]


@functools.partial(jax.pmap, in_axes=(0, 0, 0), devices=DEVICES)
def _pforward(x, x_e, p):
    return _forward_one(x, x_e, p)


def shard_inputs(inputs):
    # One image per NeuronCore, weights replicated — done once, so the
    # execution path has no cross-device resharding.
    x = np.ascontiguousarray(inputs['x'], np.float32)
    x_e = np.ascontiguousarray(inputs['x_e'], np.float32)
    xsh = jax.device_put_sharded([x[i] for i in range(NDEV)], DEVICES)
    xesh = jax.device_put_sharded([x_e[i] for i in range(NDEV)], DEVICES)
    p = {k: np.asarray(inputs[k], np.float32) for k in WEIGHT_KEYS}
    psh = jax.device_put_replicated(p, DEVICES)
    return xsh, xesh, psh


def kernel(**inputs):
    xsh, xesh, psh = shard_inputs(inputs)
    out, out_e = _pforward(xsh, xesh, psh)
    return np.asarray(out), np.asarray(out_e)


# revision 7
# speedup vs baseline: 1.9350x; 1.9350x over previous
import functools

import jax
import jax.numpy as jnp
import numpy as np

NH, WIN = 8, 7  # num_head, window
B, H, W, C = 8, 112, 112, 128
NDEV = 8


def _ln(x, w, b, eps=1e-6):
    mu = x.mean(-1, keepdims=True)
    var = ((x - mu) ** 2).mean(-1, keepdims=True)
    return (x - mu) / jnp.sqrt(var + eps) * w + b


def _dwconv_shifts(x, w49, b):
    # x: (H,W,Cc) single image, channels-last; w49: (Cc, 7, 7) depthwise; pad 3.
    # Implemented as 49 shifted adds on a zero-padded tensor so the Neuron
    # compiler sees only slices/multiplies/adds (no conv HLO).
    Hh, Ww, Cc = x.shape
    xp = jnp.pad(x, ((3, 3), (3, 3), (0, 0)))
    out = jnp.zeros_like(x)
    for dy in range(7):
        for dx in range(7):
            out = out + xp[dy:dy + Hh, dx:dx + Ww, :] * w49[:, dy, dx]
    return out + b


def _bilinear_mat(n_out, n_in):
    # Half-pixel-center bilinear interpolation matrix (n_out, n_in), matching
    # jax.image.resize(method='bilinear') for upsampling.
    m = np.zeros((n_out, n_in), np.float32)
    scale = n_in / n_out
    for i in range(n_out):
        src = (i + 0.5) * scale - 0.5
        i0 = int(np.floor(src))
        f = src - i0
        i0c = min(max(i0, 0), n_in - 1)
        i1c = min(max(i0 + 1, 0), n_in - 1)
        m[i, i0c] += 1.0 - f
        m[i, i1c] += f
    return m


_UP = jnp.asarray(_bilinear_mat(H, WIN))  # (112, 7)


def _forward_one(x, x_e, p):
    # x: (H,W,C), x_e: (H,W,C/2) — one image.
    hd = C // NH // 2
    xn = _ln(x, p['norm_w'], p['norm_b'])
    xen = _ln(x_e, p['norme_w'], p['norme_b'])

    # local_rr branch
    rr1 = xn @ p['rr1_w'] + p['rr1_b']
    rr2 = xn @ p['rr2_w'] + p['rr2_b']
    rr2 = _dwconv_shifts(rr2, p['conv_w'][:, 0], p['conv_b'])
    rr2 = rr2 @ p['rr3_w'] + p['rr3_b']
    local_rr = rr1 * rr2  # (H,W,C)

    # LocalAttentionRGBT branch
    rgb = xn @ p['la1_w'] + p['la1_b']      # (H,W,C/2)
    t = xen @ p['la2_w'] + p['la2_b']       # (H,W,C/2)
    co = _dwconv_shifts(rgb * t, p['lac1_w'][:, 0], p['lac1_b'])
    di = _dwconv_shifts(jnp.abs(rgb - t), p['lac2_w'][:, 0], p['lac2_b'])
    co_di = jnp.concatenate([co, di], -1)   # (H,W,C)
    amap = co_di.mean(-1, keepdims=True)    # (H,W,1)
    dot = (amap * co_di).sum((0, 1))        # (C,)
    n1 = jnp.sqrt((amap ** 2).sum())
    n2 = jnp.sqrt((co_di ** 2).sum((0, 1)))  # (C,)
    cos = dot / (n1 * n2 + 1e-6)            # (C,)
    attc = jax.nn.sigmoid(
        jax.nn.gelu(cos @ p['fc1_w'], approximate=False) @ p['fc2_w'])
    local_rx = (co_di * attc) @ p['la3_w'] + p['la3_b']  # (H,W,C/2)

    # global pooled attention
    kv = (xn @ p['kv_w'] + p['kv_b']).reshape(H * W, 2, NH, hd)
    k = kv[:, 0].transpose(1, 0, 2)  # (NH, HW, hd)
    v = kv[:, 1].transpose(1, 0, 2)
    rx = jnp.concatenate([xn, xen], -1)  # (H,W,3C/2)
    ph, pw = H // WIN, W // WIN
    rx_pool = rx.reshape(WIN, ph, WIN, pw, rx.shape[-1]).mean((1, 3))
    q = (rx_pool @ p['q_w'] + p['q_b']).reshape(WIN * WIN, NH, hd)
    q = q.transpose(1, 0, 2)  # (NH, 49, hd)
    logits = (q * hd ** -0.5) @ k.transpose(0, 2, 1)  # (NH, 49, HW)
    logits = logits - logits.max(-1, keepdims=True)
    e = jnp.exp(logits)
    attn = e / e.sum(-1, keepdims=True)
    g = attn @ v  # (NH, 49, hd)
    g = g.reshape(NH, WIN, WIN, hd).transpose(0, 3, 1, 2).reshape(C // 2, WIN, WIN)
    # bilinear 7->112 as two small matmuls: (C/2,112,112)
    g = jnp.einsum('ha,cab,wb->hwc', _UP, g, _UP)

    xc = jnp.concatenate([local_rr, g, local_rx], -1)  # (H,W,2C)
    out_e = xc @ p['proje_w'] + p['proje_b']
    out = xc @ p['proj_w'] + p['proj_b']
    return out, out_e


WEIGHT_KEYS = [
    'norm_w', 'norm_b', 'norme_w', 'norme_b',
    'rr1_w', 'rr1_b', 'rr2_w', 'rr2_b', 'rr3_w', 'rr3_b', 'conv_w', 'conv_b',
    'la1_w', 'la1_b', 'la2_w', 'la2_b', 'lac1_w', 'lac1_b', 'lac2_w', 'lac2_b',
    'fc1_w', 'fc2_w', 'la3_w', 'la3_b',
    'kv_w', 'kv_b', 'q_w', 'q_b', 'proj_w', 'proj_b', 'proje_w', 'proje_b',
]


DEVICES = jax.devices()[:NDEV]


@functools.partial(jax.pmap, in_axes=(0, 0, 0), devices=DEVICES)
def _pforward(x, x_e, p):
    return _forward_one(x, x_e, p)


def shard_inputs(inputs):
    # One image per NeuronCore, weights replicated — done once, so the
    # execution path has no cross-device resharding.
    x = np.ascontiguousarray(inputs['x'], np.float32)
    x_e = np.ascontiguousarray(inputs['x_e'], np.float32)
    xsh = jax.device_put_sharded([x[i] for i in range(NDEV)], DEVICES)
    xesh = jax.device_put_sharded([x_e[i] for i in range(NDEV)], DEVICES)
    p = {k: np.asarray(inputs[k], np.float32) for k in WEIGHT_KEYS}
    psh = jax.device_put_replicated(p, DEVICES)
    return xsh, xesh, psh


def kernel(**inputs):
    xsh, xesh, psh = shard_inputs(inputs)
    out, out_e = _pforward(xsh, xesh, psh)
    return np.asarray(out), np.asarray(out_e)
